# revision 22
# baseline (speedup 1.0000x reference)
"""Distributed Bass kernel for nn_Attention_25297357373492 on 8 TRN2 NeuronCores.

Reference computation (B=2, N=2048, D=1024, H=16, DH=64):
  xn   = layernorm_over_seq(x) * g
  q    = xn @ wq.T  * scale ; k,v = split(xn @ wkv.T)
  sim  = q k^T + rel_pos_bias ; attn = softmax(sim)
  out  = (attn v) reshaped ; final = out @ wout.T

Sharding: tensor-parallel over heads (2 heads/core). Each core:
  - LayerNorms a 128-row d-slice of x^T, AllGather -> full xn^T [1024, 4096]
  - projects q^T,k^T,v^T for its 2 heads (128 inner dims)
  - computes S^T = k q^T per (b,h) tile-wise (scores transposed so softmax's
    reduction lands on the PE contraction axis), E = exp(S^T) * exp(bias^T)
    (exp(bias^T) precomputed on host, streamed as bf16)
  - PV with a ones-augmented V (M=65) so the softmax denominator falls out of
    the same matmul; normalization via reciprocal + K=1 broadcast matmul
  - AllToAll redistributes O^T from head-sharded to seq-sharded
  - final projection: out^T[:, my 512 cols] = wout @ O^T slice
Host concatenates the 8 column slices and transposes back.

All f32 matmuls run as float32r (full-rate PE mode; storage identical to f32).
"""

import numpy as np
import ml_dtypes

from concourse import bass, bacc, tile, mybir
from concourse.bass_utils import run_bass_kernel_spmd
from concourse.masks import make_identity

F32 = mybir.dt.float32
F32R = mybir.dt.float32r
BF16 = mybir.dt.bfloat16

B, N, D, H, DH = 2, 2048, 1024, 16, 64
BN = B * N                      # 4096
R = 8                           # cores
HL = H // R                     # 2 heads per core
EC = HL * DH                    # 128 inner dims per core
SCALE = DH ** -0.5
EPS = 1e-5
AX = mybir.AxisListType
ALU = mybir.AluOpType
AF = mybir.ActivationFunctionType
RG = [list(range(R))]


def build_nc():
    nc = bacc.Bacc("TRN2", target_bir_lowering=False, debug=False,
                   num_devices=R)

    xt = nc.declare_dram_parameter("xt", [128, BN], F32, isOutput=False)
    xtf = nc.declare_dram_parameter("xtf", [D, BN], F32R, isOutput=False)
    gsh = nc.declare_dram_parameter("gsh", [128, 1], F32, isOutput=False)
    wqt = nc.declare_dram_parameter("wqt", [D, EC], F32R, isOutput=False)
    wkt = nc.declare_dram_parameter("wkt", [D, EC], F32R, isOutput=False)
    wvt = nc.declare_dram_parameter("wvt", [D, EC], F32R, isOutput=False)
    wot = nc.declare_dram_parameter("wot", [D, D], BF16, isOutput=False)
    eb = nc.declare_dram_parameter("eb", [HL, N, N], BF16, isOutput=False)
    out_ext = nc.declare_dram_parameter("out", [D, BN // R], F32, isOutput=True)

    with tile.TileContext(nc) as tc:
        with tc.tile_pool(name="dram", bufs=1, space="DRAM") as dram, \
             tc.tile_pool(name="persist", bufs=1) as pp:
            st_sh = dram.tile([128, 4], F32)
            st_all = dram.tile([D, 4], F32, addr_space="Shared")
            o_sh = dram.tile([D, BN // R], BF16)
            o_a2a = dram.tile([D, BN // R], BF16)

            # ---------------- Phase 0: LN statistics on our d-slice ------
            with tc.tile_pool(name="ln", bufs=1) as ln, \
                 tc.tile_pool(name="lnst", bufs=1) as lnst:
                x_sb = ln.tile([128, BN], F32)
                nc.sync.dma_start(out=x_sb[:], in_=xt[:, :])
                g_sb = lnst.tile([128, 1], F32)
                nc.sync.dma_start(out=g_sb[:], in_=gsh[:, :])
                sq_scr = ln.tile([128, N], F32)
                st_sb = lnst.tile([128, 4], F32)
                for b in range(B):
                    half = x_sb[:, b * N:(b + 1) * N]
                    s1 = lnst.tile([128, 1], F32, tag="s1", bufs=2)
                    nc.vector.tensor_reduce(s1[:], half, AX.X, ALU.add)
                    sq = lnst.tile([128, 1], F32, tag="sq", bufs=2)
                    nc.scalar.activation(sq_scr[:], half, AF.Square,
                                         accum_out=sq[:])
                    mean = lnst.tile([128, 1], F32, tag="mean", bufs=2)
                    nc.vector.tensor_scalar_mul(mean[:], s1[:], 1.0 / N)
                    var = lnst.tile([128, 1], F32, tag="var", bufs=2)
                    nc.vector.tensor_scalar_mul(var[:], sq[:], 1.0 / N)
                    m2 = lnst.tile([128, 1], F32, tag="m2", bufs=2)
                    nc.vector.tensor_mul(m2[:], mean[:], mean[:])
                    nc.vector.tensor_tensor(var[:], var[:], m2[:], ALU.subtract)
                    nc.vector.tensor_scalar_max(var[:], var[:], EPS)
                    sd = lnst.tile([128, 1], F32, tag="sd", bufs=2)
                    nc.scalar.activation(sd[:], var[:], AF.Sqrt)
                    rstd = lnst.tile([128, 1], F32, tag="rstd", bufs=2)
                    nc.vector.reciprocal(rstd[:], sd[:])
                    nc.vector.tensor_mul(st_sb[:, b:b + 1], rstd[:], g_sb[:])
                    nc.vector.tensor_mul(st_sb[:, 2 + b:3 + b], mean[:],
                                         st_sb[:, b:b + 1])
                nc.sync.dma_start(out=st_sh[:], in_=st_sb[:])
            nc.gpsimd.collective_compute(
                "AllGather", ALU.bypass, ins=[st_sh.opt()],
                outs=[st_all.opt()], replica_groups=RG)

            # persistent weights / identity / ones
            wq_sb = pp.tile([128, 8 * EC], F32R, tag="wq", name="wq_sb")
            wk_sb = pp.tile([128, 8 * EC], F32R, tag="wk", name="wk_sb")
            wv_sb = pp.tile([128, 8 * EC], F32R, tag="wv", name="wv_sb")
            wt_sb = pp.tile([128, 8 * D], BF16, tag="wt", name="wt_sb")
            for ecb in range(8):
                nc.gpsimd.dma_start(out=wq_sb[:, ecb * EC:(ecb + 1) * EC],
                                    in_=wqt[ecb * 128:(ecb + 1) * 128, :])
                nc.gpsimd.dma_start(out=wk_sb[:, ecb * EC:(ecb + 1) * EC],
                                    in_=wkt[ecb * 128:(ecb + 1) * 128, :])
                nc.gpsimd.dma_start(out=wv_sb[:, ecb * EC:(ecb + 1) * EC],
                                    in_=wvt[ecb * 128:(ecb + 1) * 128, :])
                nc.gpsimd.dma_start(out=wt_sb[:, ecb * D:(ecb + 1) * D],
                                    in_=wot[ecb * 128:(ecb + 1) * 128, :])
            sta_sb = pp.tile([128, 32], F32, tag="sta", name="sta_sb")
            for ecb in range(8):
                nc.sync.dma_start(out=sta_sb[:, ecb * 4:(ecb + 1) * 4],
                                  in_=st_all[ecb * 128:(ecb + 1) * 128, :])
            wmod = {}
            for wname, wsb in (("q", wq_sb), ("k", wk_sb), ("v", wv_sb)):
                for b in range(B):
                    m = pp.tile([128, 8 * EC], F32R, tag=f"wm{wname}{b}",
                                name=f"wm{wname}{b}")
                    wmod[(wname, b)] = m
                    for ecb in range(8):
                        nc.vector.tensor_scalar_mul(
                            m[:, ecb * EC:(ecb + 1) * EC],
                            wsb[:, ecb * EC:(ecb + 1) * EC],
                            sta_sb[:, ecb * 4 + b:ecb * 4 + b + 1])
            csb = {}
            with tc.tile_pool(name="cps", bufs=2, space="PSUM") as cpp:
                for wname, wsb in (("q", wq_sb), ("k", wk_sb), ("v", wv_sb)):
                    cp = cpp.tile([128, 2], F32, tag="cp")
                    for ecb in range(8):
                        nc.tensor.matmul(
                            cp[:],
                            wsb[:, ecb * EC:(ecb + 1) * EC],
                            sta_sb[:, ecb * 4 + 2:ecb * 4 + 4].bitcast(F32R),
                            start=(ecb == 0), stop=(ecb == 7))
                    c = pp.tile([128, 2], F32, tag=f"c{wname}",
                                name=f"c{wname}")
                    csb[wname] = c
                    nc.vector.tensor_scalar_mul(c[:], cp[:], -1.0)
            ident = pp.tile([128, 128], F32, tag="ident", name="ident")
            make_identity(nc, ident[:])
            ones64f = pp.tile([1, 64], F32, tag="ones64f", name="ones64f")
            nc.vector.memset(ones64f[:], 1.0)
            ones64 = pp.tile([1, 64], F32R, tag="ones64", name="ones64")
            nc.scalar.copy(ones64[:], ones64f[:])

            # ---------------- Phase 1: q/k/v projections -----------------
            qT = pp.tile([128, BN], F32R, tag="qT", name="qT")
            kT = pp.tile([128, BN], F32R, tag="kT", name="kT")
            vT = pp.tile([128, BN], F32, tag="vT", name="vT")
            with tc.tile_pool(name="xnc", bufs=10) as xnp, \
                 tc.tile_pool(name="pps", bufs=3, space="PSUM") as pps:
                for cp_ in range(4):  # bn chunk-pairs of 1024
                    b = cp_ // 2
                    xc = []
                    for ecb in range(8):
                        t = xnp.tile([128, 1024], F32R, tag="xc")
                        nc.sync.dma_start(
                            out=t[:],
                            in_=xtf[ecb * 128:(ecb + 1) * 128,
                                    cp_ * 1024:(cp_ + 1) * 1024])
                        xc.append(t)
                    for wname, dst in (("v", vT), ("k", kT), ("q", qT)):
                        w = wmod[(wname, b)]
                        ps = pps.tile([128, 1024], F32, tag="pps")
                        for c2 in range(2):
                            for ecb in range(8):
                                nc.tensor.matmul(
                                    ps[:, c2 * 512:(c2 + 1) * 512],
                                    w[:, ecb * EC:(ecb + 1) * EC],
                                    xc[ecb][:, c2 * 512:(c2 + 1) * 512],
                                    start=(ecb == 0), stop=(ecb == 7))
                        dstap = dst[:, cp_ * 1024:(cp_ + 1) * 1024]
                        if wname == "k":
                            nc.vector.tensor_scalar_add(
                                dstap, ps[:], csb[wname][:, b:b + 1])
                        else:
                            nc.scalar.activation(
                                dstap, ps[:], AF.Identity,
                                bias=csb[wname][:, b:b + 1], scale=1.0)

            # ---------------- Phase 2: build ones-augmented V ------------
            va = [pp.tile([128, 16, 65], BF16, tag=f"va{bh}", name=f"va{bh}")
                  for bh in range(B * HL)]
            with tc.tile_pool(name="vtp", bufs=2, space="PSUM") as vtp:
                for b in range(B):
                    for hl in range(HL):
                        bh = b * HL + hl
                        nc.vector.memset(va[bh][:, :, 64], 1.0)
                        for jt in range(16):
                            vp = vtp.tile([128, 64], F32, tag="vp")
                            nc.tensor.transpose(
                                vp[:],
                                vT[hl * 64:(hl + 1) * 64,
                                   b * N + jt * 128: b * N + (jt + 1) * 128],
                                ident[hl * 64:(hl + 1) * 64,
                                      hl * 64:(hl + 1) * 64])
                            nc.vector.tensor_copy(va[bh][:, jt, 0:64], vp[:])

            # ---------------- Phase 3: attention, hl outer / b inner ------
            with tc.tile_pool(name="sps", bufs=2, space="PSUM") as sps, \
                 tc.tile_pool(name="pvps", bufs=2, space="PSUM") as pvps, \
                 tc.tile_pool(name="ebp", bufs=4) as ebp, \
                 tc.tile_pool(name="ep", bufs=4) as ep, \
                 tc.tile_pool(name="op", bufs=2) as op_pool, \
                 tc.tile_pool(name="rcp", bufs=2) as rcp:
                for hl in range(HL):
                    for ih in range(2):  # i-halves within each batch
                        pvs = [pvps.tile([128, 1024], F32, tag="pv",
                                         name=f"pv{hl}_{ih}_{b}")
                               for b in range(B)]
                        for jt in range(16):
                            eb_sb = ebp.tile([128, 1024], BF16, tag="eb")
                            nc.sync.dma_start(
                                out=eb_sb[:],
                                in_=eb[hl, jt * 128:(jt + 1) * 128,
                                       ih * 1024:(ih + 1) * 1024])
                            for b in range(B):
                                bh = b * HL + hl
                                kT_h = kT[hl * 64:(hl + 1) * 64,
                                          b * N:(b + 1) * N]
                                qT_h = qT[hl * 64:(hl + 1) * 64,
                                          b * N:(b + 1) * N]
                                s_ps = sps.tile([128, 1024], F32, tag="s")
                                for c2 in range(2):
                                    nc.tensor.matmul(
                                        s_ps[:, c2 * 512:(c2 + 1) * 512],
                                        kT_h[:, jt * 128:(jt + 1) * 128],
                                        qT_h[:, ih * 1024 + c2 * 512:
                                             ih * 1024 + (c2 + 1) * 512],
                                        start=True, stop=True)
                                e_sb = ep.tile([128, 1024], BF16, tag="e")
                                nc.scalar.activation(e_sb[:], s_ps[:], AF.Exp)
                                nc.vector.tensor_mul(e_sb[:], e_sb[:],
                                                     eb_sb[:])
                                for c2 in range(2):
                                    nc.tensor.matmul(
                                        pvs[b][0:65,
                                               c2 * 512:(c2 + 1) * 512],
                                        va[bh][:, jt, :],
                                        e_sb[:, c2 * 512:(c2 + 1) * 512],
                                        start=(jt == 0), stop=(jt == 15))
                        for b in range(B):
                            pv = pvs[b]
                            rec = rcp.tile([1, 1024], F32R, tag="rec")
                            with nc.allow_low_precision(
                                    reason="f32r rec feeds f32r bcast mm"):
                                nc.vector.reciprocal(rec[:], pv[64:65, :])
                            bc = sps.tile([64, 1024], F32, tag="s")
                            for c2 in range(2):
                                nc.tensor.matmul(
                                    bc[:, c2 * 512:(c2 + 1) * 512],
                                    ones64[:],
                                    rec[:, c2 * 512:(c2 + 1) * 512],
                                    start=True, stop=True)
                            bc_sb = op_pool.tile([64, 1024], F32, tag="bcs")
                            nc.vector.tensor_copy(bc_sb[:], bc[:])
                            o_sb = op_pool.tile([64, 1024], BF16, tag="o")
                            nc.vector.tensor_mul(o_sb[:], pv[0:64, :],
                                                 bc_sb[:])
                            base = b * N + ih * 1024
                            for c2 in range(2):
                                s_idx = (base + c2 * 512) // 512
                                nc.gpsimd.dma_start(
                                    out=o_sh[s_idx * 128 + hl * 64:
                                             s_idx * 128 + hl * 64 + 64, :],
                                    in_=o_sb[:, c2 * 512:(c2 + 1) * 512])

            nc.gpsimd.collective_compute(
                "AllToAll", ALU.bypass, ins=[o_sh.opt()],
                outs=[o_a2a.opt()], replica_groups=RG)

            # ---------------- Phase 4: final projection ------------------
            with tc.tile_pool(name="ocp", bufs=10) as ocp, \
                 tc.tile_pool(name="fsb", bufs=2) as fsb, \
                 tc.tile_pool(name="fps", bufs=2, space="PSUM") as fps:
                oc = []
                for ecb in range(8):
                    t = ocp.tile([128, 512], BF16, tag="oc")
                    nc.gpsimd.dma_start(
                        out=t[:], in_=o_a2a[ecb * 128:(ecb + 1) * 128, :])
                    oc.append(t)
                for dt_ in range(8):
                    f_ps = fps.tile([128, 512], F32, tag="f")
                    for ecb in range(8):
                        nc.tensor.matmul(
                            f_ps[:],
                            wt_sb[:, ecb * D + dt_ * 128:
                                  ecb * D + (dt_ + 1) * 128],
                            oc[ecb][:],
                            start=(ecb == 0), stop=(ecb == 7))
                    f_sb = fsb.tile([128, 512], F32, tag="fo")
                    nc.scalar.copy(f_sb[:], f_ps[:])
                    nc.gpsimd.dma_start(
                        out=out_ext[dt_ * 128:(dt_ + 1) * 128, :], in_=f_sb[:])
    nc.compile()
    return nc


_NC_CACHE = None
LAST_RESULT = None


def kernel(x, rel_pos_bias, g, wq, wkv, wout):
    global _NC_CACHE
    x = np.asarray(x, dtype=np.float32)
    rel_pos_bias = np.asarray(rel_pos_bias, dtype=np.float32)
    g = np.asarray(g, dtype=np.float32)
    wq = np.asarray(wq, dtype=np.float32)
    wkv = np.asarray(wkv, dtype=np.float32)
    wout = np.asarray(wout, dtype=np.float32)

    xT = np.ascontiguousarray(x.transpose(2, 0, 1).reshape(D, BN))
    wqt_full = np.ascontiguousarray((wq * SCALE).T)       # [D, INNER]
    wkvT = wkv.T                                          # [D, 2*INNER]
    wot_full = np.ascontiguousarray(wout.T)               # [INNER, D]

    in_maps = []
    for r in range(R):
        sl = slice(r * EC, (r + 1) * EC)
        ebr = np.exp(rel_pos_bias[0, r * HL:(r + 1) * HL].transpose(0, 2, 1))
        in_maps.append({
            "xt": np.ascontiguousarray(xT[sl]),
            "xtf": xT,
            "gsh": np.ascontiguousarray(g[sl].reshape(EC, 1)),
            "wqt": np.ascontiguousarray(wqt_full[:, sl]),
            "wkt": np.ascontiguousarray(wkvT[:, sl]),
            "wvt": np.ascontiguousarray(wkvT[:, D + r * EC: D + (r + 1) * EC]),
            "wot": wot_full.astype(ml_dtypes.bfloat16),
            "eb": np.ascontiguousarray(ebr).astype(ml_dtypes.bfloat16),
        })

    if _NC_CACHE is None:
        _NC_CACHE = build_nc()
    import os
    kwargs = {}
    if os.environ.get("BASS_KERNEL_TRACE"):
        kwargs["trace"] = True
    res = run_bass_kernel_spmd(_NC_CACHE, in_maps, core_ids=list(range(R)),
                               **kwargs)
    global LAST_RESULT
    LAST_RESULT = res
    outT = np.concatenate([np.asarray(res.results[r]["out"]) for r in range(R)],
                          axis=1)                          # [D, BN]
    return np.ascontiguousarray(outT.T).reshape(B, N, D).astype(np.float32)


if __name__ == "__main__":
    nc = build_nc()
    print("build OK; instructions:",
          sum(len(bb.instructions) for bb in nc.main_func.blocks))
